# revision 1
# baseline (speedup 1.0000x reference)
"""Embedding lookup (GroupedEmbedding == single gather) on 8 trn2 cores.

out[b, s, :] = weight[input_[b, s], :]   with input_ [8, 4096], weight [128000, 1024] f32.

Strategy: replicate the table, data-parallel over the batch dim (B == n_cores == 8).
Core b handles the 4096 indices of batch row b. On-core: gpsimd indirect DMA
(SWDGE) gathers 128 rows (4KB each) per call — one row per partition, offsets from
one SBUF index column — into SBUF; an HWDGE DMA then streams each 512KB block back
out to the contiguous [4096, 1024] DRAM output as soon as its gather lands, so the
read and write streams interleave densely across all 16 SDMA engines.

Raw bass (not Tile): the kernel is DMA-only; Tile's auto-sync emits multi-wait
DMA/drain instructions that overflow walrus' per-instruction sync-wait encoding
(DIRECT2D DMAs take 1 wait, the tail Drain <=4), and its tail barrier costs
~10us. With explicit semaphores every wait is its own engine instruction, and the
whole gathered activation set (128KB/partition) fits in SBUF so no buffer slot is
ever reused.

HW semantics of the indirect DMA (found empirically, differs from CoreSim): one
descriptor per CONTIGUOUS destination run, one offset consumed per run. So each
call uses a [P, 1] offset column and a [P, d] destination (128 descriptors).

Host-side index layout: idx_dram[p, c] = flat_idx[c*128 + p]; gather call c fills
rows_sb[:, c*d:(c+1)*d] with table rows for out rows c*128..c*128+127, making each
store a fully contiguous 512KB block.
"""

import numpy as np

import concourse.bass as bass
import concourse.mybir as mybir
from concourse.bass import IndirectOffsetOnAxis
from concourse.bass_utils import run_bass_kernel_spmd

V = 128000        # vocab rows
D = 1024          # embedding dim
B = 8             # batch (== n_cores)
S = 4096          # seq per core
P = 128           # SBUF partitions
N_CORES = 8


def build_nc(s=S, v=V, d=D):
    KT = s // P               # index columns (gather/store calls)
    assert s % P == 0

    nc = bass.Bass("TRN2")
    idx = nc.dram_tensor("idx", [P, KT], mybir.dt.int32, kind="ExternalInput")
    weight = nc.dram_tensor("weight", [v, d], mybir.dt.float32, kind="ExternalInput")
    out = nc.dram_tensor("out", [s, d], mybir.dt.float32, kind="ExternalOutput")

    from contextlib import ExitStack

    with ExitStack() as ctx:
        sem_idx = ctx.enter_context(nc.semaphore("sem_idx"))
        sem_g = [ctx.enter_context(nc.semaphore(f"sem_g{c}")) for c in range(KT)]
        sem_s = [ctx.enter_context(nc.semaphore(f"sem_s{c}")) for c in range(KT)]
        idx_sb = ctx.enter_context(nc.sbuf_tensor("idx_sb", [P, KT], mybir.dt.int32))
        rows_sb = ctx.enter_context(
            nc.sbuf_tensor("rows_sb", [P, KT * d], mybir.dt.float32)
        )

        # index load on gpsimd itself (SWDGE): no cross-engine hop before the
        # first descriptor generation
        nc.gpsimd.dma_start(idx_sb[:, :], idx[:, :]).then_inc(sem_idx, 16)
        nc.gpsimd.wait_ge(sem_idx, 16)

        for c in range(KT):
            nc.gpsimd.indirect_dma_start(
                out=rows_sb[:, c * d : (c + 1) * d],
                out_offset=None,
                in_=weight[:, :],
                in_offset=IndirectOffsetOnAxis(ap=idx_sb[:, c : c + 1], axis=0),
            ).then_inc(sem_g[c], 16)

        for c in range(KT):
            nc.sync.wait_ge(sem_g[c], 16)
            out_view = out[c * P : (c + 1) * P, :]
            nc.sync.dma_start(out_view, rows_sb[:, c * d : (c + 1) * d]).then_inc(
                sem_s[c], 16
            )

        for c in range(KT):
            nc.sync.wait_ge(sem_s[c], 16)

    return nc


def _pack_indices(flat_idx):
    """[s] int -> [P, s//P] int32: idx[p, c] = flat_idx[c*P + p]."""
    s = flat_idx.shape[0]
    return np.ascontiguousarray(
        flat_idx.reshape(s // P, P).T.astype(np.int32)
    )


_NC_CACHE = {}


def _get_nc():
    if "nc" not in _NC_CACHE:
        _NC_CACHE["nc"] = build_nc()
    return _NC_CACHE["nc"]


def kernel(input_, weight, trace=False, **run_kwargs):
    input_ = np.asarray(input_)
    weight = np.ascontiguousarray(np.asarray(weight), dtype=np.float32)
    nc = _get_nc()
    in_maps = [
        {"idx": _pack_indices(input_[b].ravel()), "weight": weight}
        for b in range(B)
    ]
    res = run_bass_kernel_spmd(
        nc, in_maps, core_ids=list(range(N_CORES)), trace=trace, **run_kwargs
    )
    out = np.stack([r["out"] for r in res.results], axis=0)  # [B, S, D]
    if trace:
        return out, res
    return out

